# revision 13
# baseline (speedup 1.0000x reference)
"""Half-Chamfer distance kernel for Trainium2 (8 NeuronCores).

Problem: prediction [4, 8192, 3], ground_truth [4, 8192, 3] (f32).
out[b] = mean_n min_m ||pred[b,n] - gt[b,m]||^2

Sharding: core c -> (batch b = c//2, N-half h = c%2). Each core computes
min over all M=8192 gt points for its 4096 prediction points, row-sums;
host combines the per-core [128] partial sums.

Device algorithm (per core):
  d2[n,m] = |x^ - y^|^2 computed exactly (fp32 accum) from fp16-quantized
  points via a K=7 fp16 matmul (fp16 streams 1 col/cycle at 2.4 GHz vs
  fp32r's effective 1.2 GHz):
    stationary rows [x0, x1, x2, 1, 1, x2h, x2l]   (pred pts on columns)
    moving rows     [-2y0, -2y1, -2y2, qh, ql, 1, 1]
  where x2h+x2l and qh+ql are hi/lo fp16 splits of |x^|^2 and |y^|^2
  (keeps the cancellation-sensitive norm terms at ~2^-22 accuracy).

  Even/odd m-columns form separate moving tensors E and O. Per 1024-wide
  chunk-pair:
    PE     -> E, O into PSUM               (4 matmuls, FD=512 each)
    ScalarE-> copy O PSUM->SBUF
    VectorE-> one fused custom-DVE op: accum = min(accum_prev,
              min_j min(E_psum[j], O_sbuf[j]))  -- 1 elem/cycle ingest of
              2 streams, NO separate tensor_reduce pass.
  The per-chunk accumulators chain via the op's seed scalar (s0 AP), so
  the 4th chunk writes d_x for the n-tile directly. Final relu + row-sum
  on device; host sums 128 partials per core.
"""

import numpy as np

import concourse.bass as bass
import concourse.mybir as mybir
from concourse.bass_utils import run_bass_kernel_spmd
from concourse.tile import TileContext

B = 4
N = 8192
M = 8192
D = 3
N_CORES = 8
N_SH = N // 2          # 4096 prediction points per core
J = M // 2             # 4096 m-pairs
KR = 7                 # contraction rows
JC = 512               # cols per matmul (1 PSUM bank of fp32)
CP = 1024              # chunk-pair width (2 matmuls per E/O chunk)
NTILES = N_SH // 128   # 32 n-tiles of 128 partitions
CHUNKS = J // CP       # 4 chunk-pairs per n-tile

F32 = mybir.dt.float32
F16 = mybir.dt.float16
SEED = 3.0e38          # scan seed for the first chunk (acts as +inf)

_CACHED_NC = None


def _build_nc():
    nc = bass.Bass()
    statx_d = nc.declare_dram_parameter("statx", [KR, N_SH], F16, isOutput=False)
    emov_d = nc.declare_dram_parameter("emov", [KR, J], F16, isOutput=False)
    omov_d = nc.declare_dram_parameter("omov", [KR, J], F16, isOutput=False)
    out_d = nc.declare_dram_parameter("out", [128, 1], F32, isOutput=True)

    with TileContext(nc) as tc:
        with (
            tc.tile_pool(name="const", bufs=1) as cpool,
            tc.tile_pool(name="osb", bufs=3) as opool,
            tc.tile_pool(name="scan", bufs=3) as spool,
            tc.tile_pool(name="ps_e", bufs=2, space="PSUM") as epool,
            tc.tile_pool(name="ps_o", bufs=2, space="PSUM") as gpool,
        ):
            statx = cpool.tile([KR, N_SH], F16, tag="statx")
            emov = cpool.tile([KR, J], F16, tag="emov")
            omov = cpool.tile([KR, J], F16, tag="omov")
            dx_all = cpool.tile([128, NTILES], F32, tag="dx")
            nc.sync.dma_start(out=statx[:], in_=statx_d[:])
            nc.sync.dma_start(out=emov[:], in_=emov_d[:])
            nc.sync.dma_start(out=omov[:], in_=omov_d[:])

            for t in range(NTILES):
                lhs = statx[:, t * 128:(t + 1) * 128]
                prev = None
                for cp in range(CHUNKS):
                    e2 = epool.tile([128, CP], F32, tag="e2")
                    o2 = gpool.tile([128, CP], F32, tag="o2")
                    for k in range(2):
                        sl = slice(cp * CP + k * JC, cp * CP + (k + 1) * JC)
                        nc.tensor.matmul(
                            out=e2[:, k * JC:(k + 1) * JC],
                            lhsT=lhs, rhs=emov[:, sl],
                            start=True, stop=True,
                        )
                    for k in range(2):
                        sl = slice(cp * CP + k * JC, cp * CP + (k + 1) * JC)
                        nc.tensor.matmul(
                            out=o2[:, k * JC:(k + 1) * JC],
                            lhsT=lhs, rhs=omov[:, sl],
                            start=True, stop=True,
                        )
                    osb = opool.tile([128, CP], F32, tag="osb")
                    nc.scalar.copy(out=osb[:], in_=o2[:])
                    # Fused ingest+reduce: running min over the free axis,
                    # state = min(min(E[j], state), O[j]); last column is the
                    # chunk min. Chained across chunks via `initial`.
                    scan = spool.tile([128, CP], F32, tag="scan")
                    nc.vector.tensor_tensor_scan(
                        out=scan[:],
                        data0=e2[:],
                        data1=osb[:],
                        initial=SEED if cp == 0 else prev,
                        op0=mybir.AluOpType.min,
                        op1=mybir.AluOpType.min,
                    )
                    prev = scan[:, CP - 1:CP]
                # collect the n-tile min (ScalarE has slack; DVE is critical)
                nc.scalar.copy(out=dx_all[:, t:t + 1], in_=prev)

            # clamp at 0 (matches reference's maximum(d2, 0) before min)
            nc.vector.tensor_scalar_max(
                out=dx_all[:], in0=dx_all[:], scalar1=0.0
            )
            dxsum = cpool.tile([128, 1], F32, tag="dxsum")
            nc.vector.tensor_reduce(
                out=dxsum[:], in_=dx_all[:],
                axis=mybir.AxisListType.X, op=mybir.AluOpType.add,
            )
            nc.sync.dma_start(out=out_d[:], in_=dxsum[:])

    # Populate .instr bytes for InstISA subclasses (TensorTensorReduce);
    # raw Bass skips this pass and walrus errors "ISA wrong length".
    mybir.codegen_inst_isa_subclasses(nc)
    _legalize_for_walrus(nc)
    return nc


def _legalize_for_walrus(nc, max_waits=1):
    """This container's walrus encodes at most one sync-wait per
    instruction (fused-LW matmuls, drains, ...) and cannot encode
    EVENT_SEMAPHORE_RANGE_CLEAR at all.  Spill extra waits onto
    standalone NoOps queued just before on the same engine, and drop the
    tail sem range-clear."""
    RANGE_CLEAR_OPCODE = 176
    for f in nc.m.functions:
        for blk in f.blocks:
            out = []
            for inst in blk.instructions:
                if (
                    type(inst).__name__ == "InstISA"
                    and getattr(inst, "isa_opcode", None) == RANGE_CLEAR_OPCODE
                ):
                    continue
                si = inst.sync_info
                if si is not None and len(si.on_wait) > max_waits:
                    waits = list(si.on_wait)
                    for w in waits[:-max_waits]:
                        out.append(mybir.InstNoOp(
                            name=nc.get_next_instruction_name(),
                            engine=inst.engine,
                            sync_info=mybir.SyncInfo(
                                on_wait=[w], on_update=[]),
                        ))
                    inst.sync_info = mybir.SyncInfo(
                        on_wait=waits[-max_waits:],
                        on_update=list(si.on_update),
                    )
                out.append(inst)
            blk.instructions = out


def _get_nc():
    global _CACHED_NC
    if _CACHED_NC is None:
        _CACHED_NC = _build_nc()
    return _CACHED_NC


def _prep_core_inputs(x, y):
    """x: [N_SH, 3] f32 pred slice; y: [M, 3] f32 gt batch.

    Quantize points to fp16; compute the squared norms of the QUANTIZED
    points in f64 and hi/lo-split them into fp16 pairs, so the matmul's
    fp32 accumulation reconstructs |x^ - y^|^2 to ~1e-5 absolute."""
    xq = x.astype(np.float16)
    yq = y.astype(np.float16)
    x64 = xq.astype(np.float64)
    y64 = yq.astype(np.float64)

    x2 = (x64 * x64).sum(-1)
    x2h = x2.astype(np.float16)
    x2l = (x2 - x2h.astype(np.float64)).astype(np.float16)

    q = (y64 * y64).sum(-1)
    qh = q.astype(np.float16)
    ql = (q - qh.astype(np.float64)).astype(np.float16)

    ones_n = np.ones(N_SH, np.float16)
    ones_m = np.ones(M, np.float16)
    m2 = (-2.0 * y64).astype(np.float16)  # exact: -2 * fp16 value

    statx = np.stack([xq[:, 0], xq[:, 1], xq[:, 2], ones_n, ones_n, x2h, x2l])
    mov = np.stack([m2[:, 0], m2[:, 1], m2[:, 2], qh, ql, ones_m, ones_m])
    emov = mov[:, 0::2]
    omov = mov[:, 1::2]
    return {
        "statx": np.ascontiguousarray(statx, dtype=np.float16),
        "emov": np.ascontiguousarray(emov, dtype=np.float16),
        "omov": np.ascontiguousarray(omov, dtype=np.float16),
    }


def kernel(prediction, ground_truth, _trace=False, _trace_kwargs=None):
    prediction = np.asarray(prediction, dtype=np.float32)
    ground_truth = np.asarray(ground_truth, dtype=np.float32)
    assert prediction.shape == (B, N, D)
    assert ground_truth.shape == (B, M, D)

    nc = _get_nc()
    in_maps = []
    for c in range(N_CORES):
        b, h = c // 2, c % 2
        x = prediction[b, h * N_SH:(h + 1) * N_SH]
        in_maps.append(_prep_core_inputs(x, ground_truth[b]))

    kw = {}
    if _trace:
        kw = {"trace": True, "trace_cores": [0]}
        if _trace_kwargs:
            kw.update(_trace_kwargs)
    res = run_bass_kernel_spmd(nc, in_maps, list(range(N_CORES)), **kw)

    out = np.zeros(B, dtype=np.float64)
    for c in range(N_CORES):
        out[c // 2] += res.results[c]["out"].astype(np.float64).sum()
    out = (out / N).astype(np.float32)
    if _trace:
        kernel.last_result = res
    return out


# revision 15
# speedup vs baseline: 1.1465x; 1.1465x over previous
"""Half-Chamfer distance kernel for Trainium2 (8 NeuronCores).

Problem: prediction [4, 8192, 3], ground_truth [4, 8192, 3] (f32).
out[b] = mean_n min_m ||pred[b,n] - gt[b,m]||^2

Sharding: core c -> (batch b = c//2, N-half h = c%2). Each core computes
min over all M=8192 gt points for its 4096 prediction points, row-sums;
host combines the per-core [128] partial sums.

Device algorithm (per core), engine-balanced for this HW where the PE
streams moving data at ~1.2 GHz regardless of dtype and PSUM can only be
drained by VectorE (1 elem/cycle via its single PSUM port) and ScalarE
(copy at ~1.09 ns/elem):

  d2[n,m] exactly from fp16-quantized points via K=7 fp16 matmuls:
    stationary rows [x0, x1, x2, 1, 1, x2h, x2l]
    moving rows     [-2y0, -2y1, -2y2, qh, ql, 1, 1]
  (x2h+x2l = |x^|^2, qh+ql = |y^|^2 hi/lo fp16 splits of the f64 norms of
  the QUANTIZED points, so PSUM = |x^-y^|^2 to ~1e-5. Keeping d2 >= 0 in
  the matmul matters: small mins stay accurate in bf16 downstream.)

  Per n-tile (128 preds), M=8192 arrives as 8 PSUM chunks [128,1024]
  (2 matmuls each). Drain split tuned to measured rates:
    - chunks 0,4: consumed directly by VectorE TT-min against a copied
      chunk (1x, 1224ns) -> bf16
    - chunks 1,2,3,5,6,7: ScalarE-copied PSUM->SBUF as bf16 (1114ns)
    - bf16 merge tree on VectorE at 2x (692ns per [128,1024] TT-min)
    - final tensor_reduce min [128,1024] -> dx column (1219ns)
  DVE ~7.1us/n-tile, ACT ~6.7us, PE ~6.8us single-stream -> PE is row-
  tiled 2x (tile_position (0,0)/(32,0), inputs replicated at partition
  offset 32) so two n-tiles' matmuls stream concurrently (~3.4us each).

Tail: relu-clamp + row-sum on device; host sums 128 partials per core.
"""

import numpy as np

import concourse.bass as bass
import concourse.mybir as mybir
from concourse.bass_utils import run_bass_kernel_spmd
from concourse.tile import TileContext

B = 4
N = 8192
M = 8192
D = 3
N_CORES = 8
N_SH = N // 2          # 4096 prediction points per core
KR = 7                 # contraction rows
JC = 512               # cols per matmul (1 PSUM bank of fp32)
CP = 1024              # chunk width (2 matmuls per chunk)
NTILES = N_SH // 128   # 32 n-tiles of 128 partitions
CHUNKS = M // CP       # 8 chunks per n-tile
PSUM_DIRECT = (0, 4)   # chunks drained by DVE straight from PSUM
COPIED = (1, 2, 3, 5, 6, 7)

F32 = mybir.dt.float32
F16 = mybir.dt.float16
BF16 = mybir.dt.bfloat16

_CACHED_NC = None


def _build_nc():
    nc = bass.Bass()
    statx_d = nc.declare_dram_parameter("statx", [KR, N_SH], F16, isOutput=False)
    mov_d = nc.declare_dram_parameter("mov", [KR, M], F16, isOutput=False)
    out_d = nc.declare_dram_parameter("out", [128, 1], F32, isOutput=True)

    with TileContext(nc) as tc:
        with (
            tc.tile_pool(name="const", bufs=1) as cpool,
            tc.tile_pool(name="cp", bufs=8) as copool,
            tc.tile_pool(name="tr", bufs=8) as trpool,
            tc.tile_pool(name="ps", bufs=4, space="PSUM") as pspool,
        ):
            # inputs replicated at partition offsets 0 and 32 (PE row bands)
            statx = cpool.tile([39, N_SH], F16, tag="statx")
            mov = cpool.tile([39, M], F16, tag="mov")
            dx_all = cpool.tile([128, NTILES], F32, tag="dx")
            nc.sync.dma_start(out=statx[0:KR, :], in_=statx_d[:])
            nc.sync.dma_start(out=statx[32:32 + KR, :], in_=statx_d[:])
            nc.sync.dma_start(out=mov[0:KR, :], in_=mov_d[:])
            nc.sync.dma_start(out=mov[32:32 + KR, :], in_=mov_d[:])

            for t in range(NTILES):
                base = 32 * (t % 2)     # PE row band for this n-tile
                lhs = statx[base:base + KR, t * 128:(t + 1) * 128]

                def mm_chunk(k):
                    p = pspool.tile([128, CP], F32, tag="ps")
                    for j in range(2):
                        sl = slice(k * CP + j * JC, k * CP + (j + 1) * JC)
                        nc.tensor.matmul(
                            out=p[:, j * JC:(j + 1) * JC],
                            lhsT=lhs, rhs=mov[base:base + KR, sl],
                            start=True, stop=True,
                            tile_position=(base, 0),
                        )
                    return p

                def tt_min(dst, a, b):
                    nc.vector.tensor_tensor(
                        out=dst, in0=a, in1=b, op=mybir.AluOpType.min
                    )

                # chunks 0,1: TT(psum0, S1)
                p0 = mm_chunk(0)
                p1 = mm_chunk(1)
                s1 = copool.tile([128, CP], BF16, tag="s")
                nc.scalar.copy(out=s1[:], in_=p1[:])
                a0 = trpool.tile([128, CP], BF16, tag="a")
                tt_min(a0[:], p0[:], s1[:])

                # chunks 2,3: copied, bf16-merged
                p2 = mm_chunk(2)
                s2 = copool.tile([128, CP], BF16, tag="s")
                nc.scalar.copy(out=s2[:], in_=p2[:])
                p3 = mm_chunk(3)
                s3 = copool.tile([128, CP], BF16, tag="s")
                nc.scalar.copy(out=s3[:], in_=p3[:])
                b0 = trpool.tile([128, CP], BF16, tag="a")
                tt_min(b0[:], s2[:], s3[:])

                # chunks 4,5: TT(psum4, S5)
                p4 = mm_chunk(4)
                p5 = mm_chunk(5)
                s5 = copool.tile([128, CP], BF16, tag="s")
                nc.scalar.copy(out=s5[:], in_=p5[:])
                a1 = trpool.tile([128, CP], BF16, tag="a")
                tt_min(a1[:], p4[:], s5[:])

                # chunks 6,7: copied, bf16-merged
                p6 = mm_chunk(6)
                s6 = copool.tile([128, CP], BF16, tag="s")
                nc.scalar.copy(out=s6[:], in_=p6[:])
                p7 = mm_chunk(7)
                s7 = copool.tile([128, CP], BF16, tag="s")
                nc.scalar.copy(out=s7[:], in_=p7[:])
                b1 = trpool.tile([128, CP], BF16, tag="a")
                tt_min(b1[:], s6[:], s7[:])

                c0 = trpool.tile([128, CP], BF16, tag="a")
                tt_min(c0[:], a0[:], a1[:])
                c1 = trpool.tile([128, CP], BF16, tag="a")
                tt_min(c1[:], b0[:], b1[:])
                dfin = trpool.tile([128, CP], BF16, tag="a")
                tt_min(dfin[:], c0[:], c1[:])
                nc.vector.tensor_reduce(
                    out=dx_all[:, t:t + 1], in_=dfin[:],
                    axis=mybir.AxisListType.X, op=mybir.AluOpType.min,
                )

            # clamp at 0 (matches reference's maximum(d2, 0) before min)
            nc.vector.tensor_scalar_max(
                out=dx_all[:], in0=dx_all[:], scalar1=0.0
            )
            dxsum = cpool.tile([128, 1], F32, tag="dxsum")
            nc.vector.tensor_reduce(
                out=dxsum[:], in_=dx_all[:],
                axis=mybir.AxisListType.X, op=mybir.AluOpType.add,
            )
            nc.sync.dma_start(out=out_d[:], in_=dxsum[:])

    # Populate .instr bytes for InstISA subclasses; this walrus errors
    # "ISA wrong length" on empty payloads.
    mybir.codegen_inst_isa_subclasses(nc)
    _legalize_for_walrus(nc)
    return nc


def _legalize_for_walrus(nc, max_waits=1):
    """This container's walrus encodes at most one sync-wait per
    instruction (fused-LW matmuls, drains, ...) and cannot encode
    EVENT_SEMAPHORE_RANGE_CLEAR at all.  Spill extra waits onto
    standalone NoOps queued just before on the same engine, and drop the
    tail sem range-clear."""
    RANGE_CLEAR_OPCODE = 176
    for f in nc.m.functions:
        for blk in f.blocks:
            out = []
            for inst in blk.instructions:
                if (
                    type(inst).__name__ == "InstISA"
                    and getattr(inst, "isa_opcode", None) == RANGE_CLEAR_OPCODE
                ):
                    continue
                si = inst.sync_info
                if si is not None and len(si.on_wait) > max_waits:
                    waits = list(si.on_wait)
                    for w in waits[:-max_waits]:
                        out.append(mybir.InstNoOp(
                            name=nc.get_next_instruction_name(),
                            engine=inst.engine,
                            sync_info=mybir.SyncInfo(
                                on_wait=[w], on_update=[]),
                        ))
                    inst.sync_info = mybir.SyncInfo(
                        on_wait=waits[-max_waits:],
                        on_update=list(si.on_update),
                    )
                out.append(inst)
            blk.instructions = out


def _get_nc():
    global _CACHED_NC
    if _CACHED_NC is None:
        _CACHED_NC = _build_nc()
    return _CACHED_NC


def _prep_core_inputs(x, y):
    """x: [N_SH, 3] f32 pred slice; y: [M, 3] f32 gt batch.

    Quantize points to fp16; compute the squared norms of the QUANTIZED
    points in f64 and hi/lo-split them into fp16 pairs, so the matmul's
    fp32 accumulation reconstructs |x^ - y^|^2 to ~1e-5 absolute."""
    xq = x.astype(np.float16)
    yq = y.astype(np.float16)
    x64 = xq.astype(np.float64)
    y64 = yq.astype(np.float64)

    x2 = (x64 * x64).sum(-1)
    x2h = x2.astype(np.float16)
    x2l = (x2 - x2h.astype(np.float64)).astype(np.float16)

    q = (y64 * y64).sum(-1)
    qh = q.astype(np.float16)
    ql = (q - qh.astype(np.float64)).astype(np.float16)

    ones_n = np.ones(N_SH, np.float16)
    ones_m = np.ones(M, np.float16)
    m2 = (-2.0 * y64).astype(np.float16)  # exact: -2 * fp16 value

    statx = np.stack([xq[:, 0], xq[:, 1], xq[:, 2], ones_n, ones_n, x2h, x2l])
    mov = np.stack([m2[:, 0], m2[:, 1], m2[:, 2], qh, ql, ones_m, ones_m])
    return {
        "statx": np.ascontiguousarray(statx, dtype=np.float16),
        "mov": np.ascontiguousarray(mov, dtype=np.float16),
    }


def kernel(prediction, ground_truth, _trace=False, _trace_kwargs=None):
    prediction = np.asarray(prediction, dtype=np.float32)
    ground_truth = np.asarray(ground_truth, dtype=np.float32)
    assert prediction.shape == (B, N, D)
    assert ground_truth.shape == (B, M, D)

    nc = _get_nc()
    in_maps = []
    for c in range(N_CORES):
        b, h = c // 2, c % 2
        x = prediction[b, h * N_SH:(h + 1) * N_SH]
        in_maps.append(_prep_core_inputs(x, ground_truth[b]))

    kw = {}
    if _trace:
        kw = {"trace": True, "trace_cores": [0]}
        if _trace_kwargs:
            kw.update(_trace_kwargs)
    res = run_bass_kernel_spmd(nc, in_maps, list(range(N_CORES)), **kw)

    out = np.zeros(B, dtype=np.float64)
    for c in range(N_CORES):
        out[c // 2] += res.results[c]["out"].astype(np.float64).sum()
    out = (out / N).astype(np.float32)
    if _trace:
        kernel.last_result = res
    return out
